# revision 8
# baseline (speedup 1.0000x reference)
"""A-Connect conv kernel for TRN2, data-parallel over batch on 8 NeuronCores.

Computation (per sample b):
    Z[b] = conv2d(X[b], W * Werr[b], SAME) + bias * Berr[b]; out = relu(Z)

Mapping: batch 32 -> 4 samples per core. Per sample the conv is 9
shifted matmuls accumulated in PSUM. The output is computed in the
zero-padded spatial geometry (64 rows x 66 cols = 4224 = 33 tiles of
128 positions): in that flattened geometry the stationary operand for
tap (dy, dx) is a single stride-1 run of the padded input at offset
q0 + dy*66 + dx, which satisfies the BIR rule that matmul operand APs
have one free dimension. The two junk columns (xp = 0, 65) are sliced
away on the host. PSUM/output tiles are [spatial, F], matching NHWC so
stores are contiguous; the per-sample bias is folded in as a K=1
matmul with a ones vector. Inputs are pre-transposed/padded on host
and fed in bf16.
"""

import numpy as np
import ml_dtypes

B, H, Wd, Cin, F, KH, KW = 32, 64, 64, 128, 256, 3, 3
NCORES = 8
BPC = B // NCORES  # samples per core
HP, WP = H + 2, Wd + 2  # zero-padded spatial
NQ = H * WP  # padded output positions per sample: 64*66 = 4224
MT = NQ // 128  # 33 M-tiles of 128 padded positions
XLEN = 4368  # 1 lead zero + 66*66 flat padded image + tail zeros
# X arrives in overlapping chunks so matmuls can start before the whole
# image is resident. Chunk k serves M-tiles [4k, 4k+4): it must cover
# flat indices [512k, 512k + 3*128 + 134 + 128) = [512k, 512k + 646).
XCH_MT = 4  # M-tiles per chunk
XCH_N = (MT + XCH_MT - 1) // XCH_MT  # 9 chunks (last one short)
XCH_LEN = XCH_MT * 128 + 134 + 128  # 774 with slack; see chunk bounds below

_compiled = None  # cached Bass program so repeated kernel() calls reuse it


def _build_bass():
    from concourse import bacc, tile, mybir

    nc = bacc.Bacc("TRN2", target_bir_lowering=False, debug=False)
    bf16 = mybir.dt.bfloat16
    f32 = mybir.dt.float32

    xp = nc.dram_tensor("xp", [BPC, Cin, XLEN], bf16, kind="ExternalInput")
    wm = nc.dram_tensor("wm", [BPC, Cin, KH * KW, F], bf16, kind="ExternalInput")
    mb = nc.dram_tensor("mb", [BPC, 128, F], f32, kind="ExternalInput")
    y = nc.dram_tensor("y", [BPC, MT, 128, F], f32, kind="ExternalOutput")

    with tile.TileContext(nc) as tc:
        with (
            tc.tile_pool(name="xpool", bufs=2) as xpool,
            tc.tile_pool(name="wpool", bufs=2) as wpool,
            tc.tile_pool(name="bpool", bufs=2) as bpool,
            tc.tile_pool(name="opool", bufs=8) as opool,
            tc.tile_pool(name="cpool", bufs=1) as cpool,
            tc.tile_pool(name="pspool", bufs=7, space="PSUM") as pspool,
            tc.tile_pool(name="wupool", bufs=1, space="PSUM") as wupool,
        ):
            # PE warmup: ~5us of dependency-free matmuls so the HAM clock
            # gate is released (K=8/8) by the time the first input DMA lands
            wu_in = cpool.tile([128, 512], bf16)
            nc.vector.memset(wu_in[:], 0.0)
            wu_ps = wupool.tile([128, 512], f32)
            for i in range(20):
                nc.tensor.matmul(
                    wu_ps[:],
                    wu_in[:, :128],
                    wu_in[:],
                    start=(i == 0),
                    stop=(i == 19),
                )
            for b in range(BPC):
                wt = wpool.tile([Cin, KH * KW, F], bf16)
                nc.sync.dma_start(wt[:], wm[b])
                xcs = []
                for k in range(XCH_N):
                    lo = 512 * k
                    ln = min(XCH_LEN, XLEN - lo)
                    xc = xpool.tile([Cin, ln], bf16, tag=f"xc{k}")
                    nc.sync.dma_start(xc[:], xp[b, :, lo : lo + ln])
                    xcs.append((lo, xc))
                bt = bpool.tile([128, F], f32)
                nc.sync.dma_start(bt[:], mb[b])
                for m in range(MT):
                    q0 = m * 128
                    lo, xc = xcs[m // XCH_MT]
                    ps = pspool.tile([128, F], f32)
                    for t in range(KH * KW):
                        dy, dx = t // KW, t % KW
                        off = q0 + dy * WP + dx - lo
                        nc.tensor.matmul(
                            ps[:],
                            xc[:, off : off + 128],
                            wt[:, t, :],
                            start=(t == 0),
                            stop=(t == 8),
                        )
                    # bias add on DVE (in PSUM), relu+copyout on ScalarE
                    nc.vector.tensor_add(ps[:], ps[:], bt[:])
                    ot = opool.tile([128, F], f32)
                    nc.scalar.activation(
                        ot[:], ps[:], mybir.ActivationFunctionType.Relu
                    )
                    nc.sync.dma_start(y[b, m], ot[:])
    nc.compile()
    return nc


def _prep_inputs(X, W, bias, Werr, Berr):
    bf16 = ml_dtypes.bfloat16
    # per-sample perturbed kernels, laid out [B, Cin, tap, F]
    memW = (W[None] * Werr).transpose(0, 3, 1, 2, 4).reshape(B, Cin, KH * KW, F)
    memW = np.ascontiguousarray(memW, dtype=bf16)
    # padded image, flattened with one lead zero so all tap offsets are >= 0
    Xpad = np.zeros((B, Cin, HP, WP), dtype=bf16)
    Xpad[:, :, 1 : H + 1, 1 : Wd + 1] = X.transpose(0, 3, 1, 2)
    Xp = np.zeros((B, Cin, XLEN), dtype=bf16)
    Xp[:, :, 1 : 1 + HP * WP] = Xpad.reshape(B, Cin, HP * WP)
    # bias broadcast across the 128 spatial partitions of an output tile
    mbias = (bias[None] * Berr).astype(np.float32)  # [B, F]
    mbias = np.ascontiguousarray(
        np.broadcast_to(mbias[:, None, :], (B, 128, F))
    )
    return Xp, memW, mbias


def _postprocess(y_cores):
    # y per core: [BPC, MT, 128, F] over padded positions (64 x 66);
    # drop the junk columns xp=0 and xp=65
    out = np.concatenate(y_cores, axis=0)  # [B, MT, 128, F]
    out = out.reshape(B, H, WP, F)[:, :, 1 : Wd + 1, :]
    return np.ascontiguousarray(out)


def kernel(X, W, bias, Werr, Berr):
    global _compiled
    from concourse.bass_utils import run_bass_kernel_spmd

    if _compiled is None:
        _compiled = _build_bass()
    nc = _compiled

    Xp, memW, mbias = _prep_inputs(X, W, bias, Werr, Berr)
    in_maps = [
        {
            "xp": Xp[c * BPC : (c + 1) * BPC],
            "wm": memW[c * BPC : (c + 1) * BPC],
            "mb": mbias[c * BPC : (c + 1) * BPC],
        }
        for c in range(NCORES)
    ]
    res = run_bass_kernel_spmd(nc, in_maps, core_ids=list(range(NCORES)))
    return _postprocess([r["y"] for r in res.results])


# revision 11
# speedup vs baseline: 1.0069x; 1.0069x over previous
"""A-Connect conv kernel for TRN2, data-parallel over batch on 8 NeuronCores.

Computation (per sample b):
    Z[b] = conv2d(X[b], W * Werr[b], SAME) + bias * Berr[b]; out = relu(Z)

Mapping: batch 32 -> 4 samples per core. Per sample the conv is 9
shifted matmuls accumulated in PSUM. The output is computed in the
zero-padded spatial geometry (64 rows x 66 cols = 4224 = 33 tiles of
128 positions): in that flattened geometry the stationary operand for
tap (dy, dx) is a single stride-1 run of the padded input at offset
q0 + dy*66 + dx, which satisfies the BIR rule that matmul operand APs
have one free dimension. The two junk columns (xp = 0, 65) are sliced
away on the host. PSUM/output tiles are [spatial, F], matching NHWC so
stores are contiguous; the per-sample bias is folded in as a K=1
matmul with a ones vector. Inputs are pre-transposed/padded on host
and fed in bf16.
"""

import numpy as np
import ml_dtypes

B, H, Wd, Cin, F, KH, KW = 32, 64, 64, 128, 256, 3, 3
NCORES = 8
BPC = B // NCORES  # samples per core
HP, WP = H + 2, Wd + 2  # zero-padded spatial
NQ = H * WP  # padded output positions per sample: 64*66 = 4224
MT = NQ // 128  # 33 M-tiles of 128 padded positions
XLEN = 4368  # 1 lead zero + 66*66 flat padded image + tail zeros
# X arrives in two overlapping chunks so matmuls can start before the
# whole image is resident: tiles 0..16 read [0, 2448); tiles 17..32
# read [2176, 4368)
XSPLIT_MT = 17
XA_END = XSPLIT_MT * 128 + 134 + 128 + 10  # 2448
XB_OFF = XSPLIT_MT * 128  # 2176

_compiled = None  # cached Bass program so repeated kernel() calls reuse it


def _build_bass():
    from concourse import bacc, tile, mybir

    nc = bacc.Bacc("TRN2", target_bir_lowering=False, debug=False)
    bf16 = mybir.dt.bfloat16
    f32 = mybir.dt.float32

    xp = nc.dram_tensor("xp", [BPC, Cin, XLEN], bf16, kind="ExternalInput")
    wm = nc.dram_tensor("wm", [BPC, Cin, KH * KW, F], bf16, kind="ExternalInput")
    mb = nc.dram_tensor("mb", [BPC, 128, F], f32, kind="ExternalInput")
    y = nc.dram_tensor("y", [BPC, MT, 128, F], f32, kind="ExternalOutput")

    with tile.TileContext(nc) as tc:
        with (
            tc.tile_pool(name="xpool", bufs=2) as xpool,
            tc.tile_pool(name="wpool", bufs=2) as wpool,
            tc.tile_pool(name="bpool", bufs=2) as bpool,
            tc.tile_pool(name="opool", bufs=8) as opool,
            tc.tile_pool(name="cpool", bufs=1) as cpool,
            tc.tile_pool(name="pspool", bufs=7, space="PSUM") as pspool,
            tc.tile_pool(name="wupool", bufs=1, space="PSUM") as wupool,
        ):
            # PE warmup: ~5us of dependency-free matmuls so the HAM clock
            # gate is released (K=8/8) by the time the first input DMA lands
            wu_in = cpool.tile([128, 512], bf16)
            nc.vector.memset(wu_in[:], 0.0)
            wu_ps = wupool.tile([128, 512], f32)
            for i in range(20):
                nc.tensor.matmul(
                    wu_ps[:],
                    wu_in[:, :128],
                    wu_in[:],
                    start=(i == 0),
                    stop=(i == 19),
                )
            for b in range(BPC):
                wt = wpool.tile([Cin, KH * KW, F], bf16)
                nc.sync.dma_start(wt[:], wm[b])
                xta = xpool.tile([Cin, XA_END], bf16, tag="xta")
                nc.sync.dma_start(xta[:], xp[b, :, :XA_END])
                bt = bpool.tile([128, F], f32)
                nc.sync.dma_start(bt[:], mb[b])
                xtb = xpool.tile([Cin, XLEN - XB_OFF], bf16, tag="xtb")
                nc.sync.dma_start(xtb[:], xp[b, :, XB_OFF:])
                for m in range(MT):
                    q0 = m * 128
                    ps = pspool.tile([128, F], f32)
                    for t in range(KH * KW):
                        dy, dx = t // KW, t % KW
                        off = q0 + dy * WP + dx
                        if m < XSPLIT_MT:
                            lhsT = xta[:, off : off + 128]
                        else:
                            lhsT = xtb[:, off - XB_OFF : off - XB_OFF + 128]
                        nc.tensor.matmul(
                            ps[:],
                            lhsT,
                            wt[:, t, :],
                            start=(t == 0),
                            stop=(t == 8),
                        )
                    # bias add on DVE (in PSUM), relu+copyout on ScalarE
                    nc.vector.tensor_add(ps[:], ps[:], bt[:])
                    ot = opool.tile([128, F], f32)
                    nc.scalar.activation(
                        ot[:], ps[:], mybir.ActivationFunctionType.Relu
                    )
                    nc.sync.dma_start(y[b, m], ot[:])
    nc.compile()
    return nc


def _prep_inputs(X, W, bias, Werr, Berr):
    bf16 = ml_dtypes.bfloat16
    X, W, bias, Werr, Berr = (
        np.asarray(a) for a in (X, W, bias, Werr, Berr)
    )
    # per-sample perturbed kernels, laid out [B, Cin, tap, F]
    memW = (W[None] * Werr).transpose(0, 3, 1, 2, 4).reshape(B, Cin, KH * KW, F)
    memW = np.ascontiguousarray(memW, dtype=bf16)
    # padded image, flattened with one lead zero so all tap offsets are >= 0
    Xpad = np.zeros((B, Cin, HP, WP), dtype=bf16)
    Xpad[:, :, 1 : H + 1, 1 : Wd + 1] = X.transpose(0, 3, 1, 2)
    Xp = np.zeros((B, Cin, XLEN), dtype=bf16)
    Xp[:, :, 1 : 1 + HP * WP] = Xpad.reshape(B, Cin, HP * WP)
    # bias broadcast across the 128 spatial partitions of an output tile
    mbias = (bias[None] * Berr).astype(np.float32)  # [B, F]
    mbias = np.ascontiguousarray(
        np.broadcast_to(mbias[:, None, :], (B, 128, F))
    )
    return Xp, memW, mbias


def _postprocess(y_cores):
    # y per core: [BPC, MT, 128, F] over padded positions (64 x 66);
    # drop the junk columns xp=0 and xp=65
    out = np.concatenate(y_cores, axis=0)  # [B, MT, 128, F]
    out = out.reshape(B, H, WP, F)[:, :, 1 : Wd + 1, :]
    return np.ascontiguousarray(out)


def kernel(X, W, bias, Werr, Berr):
    global _compiled
    from concourse.bass_utils import run_bass_kernel_spmd

    if _compiled is None:
        _compiled = _build_bass()
    nc = _compiled

    Xp, memW, mbias = _prep_inputs(X, W, bias, Werr, Berr)
    in_maps = [
        {
            "xp": Xp[c * BPC : (c + 1) * BPC],
            "wm": memW[c * BPC : (c + 1) * BPC],
            "mb": mbias[c * BPC : (c + 1) * BPC],
        }
        for c in range(NCORES)
    ]
    res = run_bass_kernel_spmd(nc, in_maps, core_ids=list(range(NCORES)))
    return _postprocess([r["y"] for r in res.results])
